# revision 93
# baseline (speedup 1.0000x reference)
"""Trainium2 Bass kernel for nn_PredictSpanSet.

Math (per batch b):
  score_j[t] = sen_vecs[b,t,:] . (W_j.T @ x[b]) + b_j . x[b]        (j in {b,i,o})
  (the reference's [bt,dim]x[dim,dim] matmul collapses to a per-batch
   vector because the hidden h is immediately dotted with x[b])
  masked scores -> lse[t] = logsumexp_j score_j[t]
  A[s] = sb[s] + cum_excl(lse)[s] - cum_excl(si)[s] - full
  B[e] = cum_excl(si)[e] + (full - cum_incl(lse)[e]) + so[e]
  out[s,e] = A[s] + B[e]  if e >= s else -1000

Device mapping (per core, 4 batches):
  - wvT = W.T @ x chunks via PE matmuls (K=128 accumulation)
  - scores via PE: lhsT = sv^T tile [d=128, t=128], rhs = wv [d=128, 3]
  - logsumexp/cumsum/carry/A-B runs batched per PAIR of batches (wide
    [128, 2*8]-shaped ops); cumsums via one PE matmul with an
    upper-triangular ones lhsT; carries via log-shift adds on a [1, *]
    row plus a K=1 ones broadcast matmul
  - span tiles: A[s] is a per-partition scalar, B[e] is broadcast across
    partitions once per batch (K=1 ones matmul into PSUM); each output
    chunk is ONE fused op: DVE tensor_scalar add or ACT activation
    Identity with per-partition bias
  - the constant -1000 left region is stored straight from a const tile,
    issued mid-kernel when the DMA queues are otherwise idle
  - all DMA traffic is spread over the sync/SP, scalar/ACT and
    gpsimd/SWDGE queues

Sharding: data-parallel over batch (32 -> 8 cores x 4), params replicated.
"""

import numpy as np

B_LOC = 4          # batches per core
GB = 2             # mid-phase group size (batches per group)
N_CORES = 8
BS = 32
BT = 1024
DIM = 512
P = 128
NCH = BT // P      # 8 chunks of 128 along t
NDCH = DIM // P    # 4 chunks of 128 along d
NEG_MASK = -1000000.0
NEG_SPAN = -1000.0


def _import_concourse():
    try:
        import concourse.bass  # noqa: F401
    except ImportError:
        import sys
        for p in ("/opt/trn_rl_repo", "/root/.axon_site/_ro/trn_rl_repo"):
            if p not in sys.path:
                sys.path.append(p)


def build_bass():
    """Build the single-core Bass program (SPMD: same program on all cores)."""
    _import_concourse()
    import concourse.bass as bass
    import concourse.bacc as bacc
    import concourse.mybir as mybir
    from concourse import tile
    from concourse.tile_rust import add_dep_helper

    f32 = mybir.dt.float32
    i32 = mybir.dt.int32
    Alu = mybir.AluOpType
    Act = mybir.ActivationFunctionType
    PSUM = bass.MemorySpace.PSUM

    # Bacc (not Bass): its compile() splits sync waits into event-semaphore
    # instructions — HW allows at most 1 wait per instruction.
    nc = bacc.Bacc()

    svt_d = nc.dram_tensor("svt", [B_LOC, DIM, BT], f32, kind="ExternalInput")
    xT_d = nc.dram_tensor("xT", [DIM, B_LOC], f32, kind="ExternalInput")
    W_d = [
        nc.dram_tensor(n, [DIM, DIM], f32, kind="ExternalInput")
        for n in ("Wb", "Wi", "Wo")
    ]
    b3_d = nc.dram_tensor("b3", [DIM, 3], f32, kind="ExternalInput")
    mask_d = nc.dram_tensor("mask", [B_LOC, BT], i32, kind="ExternalInput")
    triu_d = nc.dram_tensor("triu", [P, P], f32, kind="ExternalInput")
    trili_d = nc.dram_tensor("trili", [P, P], i32, kind="ExternalInput")
    ident_d = nc.dram_tensor("ident", [P, P], f32, kind="ExternalInput")
    out_d = nc.dram_tensor("out", [B_LOC, BT, BT], f32, kind="ExternalOutput")

    with tile.TileContext(nc) as tc:
        with tc.tile_pool(name="const", bufs=1) as cpool:
            triu_sb = cpool.tile([P, P], f32)       # upper-tri ones (incl diag)
            trili_sb = cpool.tile([P, P], i32)      # strict lower mask (int)
            ident_sb = cpool.tile([P, P], f32)
            nc.sync.dma_start(out=triu_sb[:, :], in_=triu_d[:, :])
            nc.sync.dma_start(out=trili_sb[:, :], in_=trili_d[:, :])
            nc.sync.dma_start(out=ident_sb[:, :], in_=ident_d[:, :])

            ones_row = cpool.tile([1, P], f32)
            nc.vector.memset(ones_row[:, :], 1.0)
            ones_col = cpool.tile([P, 1], f32)
            nc.vector.memset(ones_col[:, :], 1.0)

            neg128 = cpool.tile([P, P], f32)        # -1000 diag restore src
            nc.vector.memset(neg128[:, :], NEG_SPAN)
            negm_t = cpool.tile([P, B_LOC, NCH], f32)
            nc.vector.memset(negm_t[:, :, :], NEG_MASK)
            zero_t = cpool.tile([P, B_LOC, NCH], f32)
            nc.vector.memset(zero_t[:, :, :], 0.0)

            xT_sb = cpool.tile([P, NDCH, B_LOC], f32)   # x^T, e-chunked
            nc.sync.dma_start(
                out=xT_sb[:, :, :],
                in_=xT_d[:, :].rearrange("(k p) b -> p k b", p=P),
            )
            b3_sb = cpool.tile([P, NDCH, 3], f32)
            nc.sync.dma_start(
                out=b3_sb[:, :, :],
                in_=b3_d[:, :].rearrange("(k p) j -> p k j", p=P),
            )
            # mask as [p, b, c] (t = c*128 + p)
            mask_sb = cpool.tile([P, B_LOC, NCH], i32)
            nc.sync.dma_start(
                out=mask_sb[:, :, :],
                in_=mask_d[:, :].rearrange("b (c p) -> p b c", p=P),
            )
            m0f = cpool.tile([P, B_LOC, NCH], i32)  # 1 where mask==0 (int mask)
            nc.vector.tensor_scalar(
                m0f[:, :, :], mask_sb[:, :, :], 0, None, op0=Alu.is_equal
            )

            # -1000 source for the constant left region stores
            negC = cpool.tile([P, (NCH - 1) * P], f32)
            nc.vector.memset(negC[:, :], NEG_SPAN)

            # per-s-chunk staging tiles, double buffered over batches. For
            # c < 4 the tile is full width with the -1000 left region
            # prefilled (one store covers everything); for c >= 4 the tile
            # holds only [diag | right] and the big constant left region is
            # stored separately. All -1000 prefill (incl the diag block's
            # lower triangle) is written once; the per-b diag writer
            # copy_predicated never touches masked positions.
            osb_t = []
            for g in range(2):
                row = []
                for c in range(NCH):
                    t = cpool.tile([P, BT - c * P], f32, name=f"osb{g}_{c}",
                                   tag=f"osb{g}_{c}")
                    nc.gpsimd.memset(t[:, 0:P], NEG_SPAN)
                    row.append(t)
                osb_t.append(row)

            # wvT[d, m, b, j] = sum_e W_j[e, d] x[b, e]
            wT_sb = cpool.tile([P, NDCH, B_LOC, 3], f32)
            crow_sb = cpool.tile([1, B_LOC * 3], f32)  # bias dots c[b,j]
            # masked scores, all local batches: [p, b, c, j]
            scm = cpool.tile([P, B_LOC, NCH, 3], f32)
            # A/B assembly target: [p, b, series(A,ones,B), c]
            # (series 1 is unused here but keeps transposed-pair layouts)
            ab = cpool.tile([P, B_LOC, 3, NCH], f32)

            with (
                tc.tile_pool(name="svload", bufs=4) as svpool,
                tc.tile_pool(name="scps", bufs=2, space=PSUM) as scps,
                tc.tile_pool(name="mid", bufs=2) as mpool,
                tc.tile_pool(name="midps", bufs=1, space=PSUM) as mps,
                tc.tile_pool(name="rowps", bufs=1, space=PSUM) as rps,
                tc.tile_pool(name="rows", bufs=2) as rpool,
                tc.tile_pool(name="spanps", bufs=2, space=PSUM) as sps,
            ):
                # prefetch ALL sentence vectors up front, spread over queues
                sv_q = [nc.sync, nc.gpsimd, nc.scalar, nc.sync]
                svm_t = []
                for b in range(B_LOC):
                    t = svpool.tile([P, NDCH, BT], f32, name="svm", tag="svm")
                    sv_q[b].dma_start(
                        out=t[:, :, :],
                        in_=svt_d[b].rearrange("(m p) t -> p m t", p=P),
                    )
                    svm_t.append(t)

                w_q = [nc.gpsimd, nc.scalar, nc.gpsimd]
                for j in range(3):
                    W_sb = svpool.tile(
                        [P, NDCH, DIM], f32, name="W_sb", tag="W_sb", bufs=2
                    )
                    w_q[j].dma_start(
                        out=W_sb[:, :, :],
                        in_=W_d[j][:, :].rearrange("(k p) d -> p k d", p=P),
                    )
                    for m in range(NDCH):
                        pw = scps.tile([P, 16], f32, name="pw", tag="pscb")
                        for k in range(NDCH):
                            nc.tensor.matmul(
                                pw[:, 0:B_LOC],
                                W_sb[:, k, m * P:(m + 1) * P],
                                xT_sb[:, k, :],
                                start=(k == 0),
                                stop=(k == NDCH - 1),
                            )
                        nc.vector.tensor_copy(wT_sb[:, m, :, j], pw[:, 0:B_LOC])
                # bias dots: crow[0, b*3+j] = sum_e x[b,e] b3[e,j]
                cps = scps.tile([P, 16], f32, name="cps", tag="pscb")
                for b in range(B_LOC):
                    for k in range(NDCH):
                        nc.tensor.matmul(
                            cps[0:1, b * 3:(b + 1) * 3],
                            xT_sb[:, k, b:b + 1],
                            b3_sb[:, k, :],
                            start=(k == 0),
                            stop=(k == NDCH - 1),
                        )
                nc.vector.tensor_copy(crow_sb[:, :], cps[0:1, 0:B_LOC * 3])

                # ---- phase 1: scores for every local batch ----
                sel_last = {}
                for b in range(B_LOC):
                    svm = svm_t[b]
                    # scores [t, c, j] (j padded to 4 for 8B-aligned psum APs)
                    pscb = scps.tile([P, NCH, 4], f32, name="pscb", tag="pscb")
                    for c in range(NCH):
                        for m in range(NDCH):
                            nc.tensor.matmul(
                                pscb[:, c, 0:3],
                                svm[:, m, c * P:(c + 1) * P],
                                wT_sb[:, m, b, :],
                                start=(m == 0),
                                stop=False,
                            )
                        nc.tensor.matmul(
                            pscb[:, c, 0:3],
                            ones_row[:, :],
                            crow_sb[0:1, b * 3:(b + 1) * 3],
                            start=False,
                            stop=True,
                        )
                    # mask + drain psum to the wide SBUF score tensor
                    for j, fill in ((0, negm_t), (1, negm_t), (2, zero_t)):
                        sel = nc.vector.select(
                            scm[:, b, :, j], m0f[:, b, :], fill[:, b, :],
                            pscb[:, :, j],
                        )
                    sel_last[b] = sel

                # constant left-region stores: no data deps, so issue them
                # now — the scheduler slots them into queue idle time
                # delay the const stores behind each batch's score phase so
                # the scheduler doesn't run them while the input loads are
                # still contending for the queues
                neg_q = [nc.sync, nc.gpsimd, nc.sync, nc.gpsimd, nc.scalar]
                nq = 0
                for b in range(B_LOC):
                    for c in range(1, NCH):
                        di = neg_q[nq % 5].dma_start(
                            out=out_d[b, c * P:(c + 1) * P, 0:c * P],
                            in_=negC[:, 0:c * P],
                        )
                        add_dep_helper(
                            di.ins, sel_last[b].ins, sync=False,
                            reason="const stores after batch scores",
                        )
                        nq += 1

                out_q = [
                    nc.sync, nc.gpsimd, nc.scalar, nc.sync,
                    nc.gpsimd, nc.sync, nc.gpsimd, nc.scalar,
                ]
                W2 = 2 * GB * NCH  # 32
                for g in range(B_LOC // GB):
                    b0 = g * GB
                    bs = slice(b0, b0 + GB)
                    # ---- phase 2 (per group): lse / cumsums / A,B ----
                    mx = mpool.tile([P, GB, NCH], f32, name="mx", tag="mx")
                    nc.vector.reduce_max(
                        mx[:, :, :], scm[:, bs, :, :], axis=mybir.AxisListType.X
                    )
                    sh = mpool.tile([P, GB, NCH, 3], f32, name="sh", tag="sh")
                    for j in range(3):
                        nc.vector.tensor_sub(
                            sh[:, :, :, j], scm[:, bs, :, j], mx[:, :, :]
                        )
                    ex = mpool.tile([P, GB, NCH, 3], f32, name="ex", tag="ex")
                    nc.scalar.activation(ex[:, :, :, :], sh[:, :, :, :], Act.Exp)
                    ssum = mpool.tile([P, GB, NCH], f32, name="ssum", tag="ssum")
                    nc.vector.reduce_sum(
                        ssum[:, :, :], ex[:, :, :, :], axis=mybir.AxisListType.X
                    )
                    # cum_rhs: [p, series(lse,si), gb, c]
                    cum_rhs = mpool.tile(
                        [P, 2, GB, NCH], f32, name="cum_rhs", tag="cum_rhs"
                    )
                    nc.scalar.activation(
                        cum_rhs[:, 0, :, :], ssum[:, :, :], Act.Ln
                    )
                    nc.vector.tensor_add(
                        cum_rhs[:, 0, :, :], cum_rhs[:, 0, :, :], mx[:, :, :]
                    )
                    nc.vector.tensor_copy(cum_rhs[:, 1, :, :], scm[:, bs, :, 1])

                    # one psum bank: [0:32] within-chunk cumsums,
                    # [32:64] carry broadcast, [64:96] chunk totals (row 0)
                    midp = mps.tile([P, 96], f32, name="midp", tag="midp")
                    nc.tensor.matmul(
                        midp[:, 0:W2], triu_sb[:, :],
                        cum_rhs[:, :, :, :].rearrange("p a b c -> p (a b c)"),
                        start=True, stop=True,
                    )
                    nc.tensor.matmul(
                        midp[0:1, 64:64 + W2], ones_col[:, :],
                        cum_rhs[:, :, :, :].rearrange("p a b c -> p (a b c)"),
                        start=True, stop=True,
                    )
                    # inclusive scan of chunk totals (log-shift adds, [1,32])
                    r = mpool.tile([1, 2, GB, NCH], f32, name="r0", tag="rn")
                    nc.vector.tensor_copy(
                        r[:, :, :, :],
                        midp[0:1, 64:64 + W2].rearrange(
                            "p (a b c) -> p a b c", a=2, b=GB
                        ),
                    )
                    for s in (1, 2, 4):
                        rn = mpool.tile(
                            [1, 2, GB, NCH], f32, name=f"rn{s}", tag="rn"
                        )
                        nc.vector.tensor_add(
                            rn[:, :, :, s:NCH], r[:, :, :, s:NCH],
                            r[:, :, :, 0:NCH - s],
                        )
                        nc.vector.tensor_copy(rn[:, :, :, 0:s], r[:, :, :, 0:s])
                        r = rn
                    # carry_A = excl_lse - excl_si - full[b]; carry_B = -carry_A
                    carry = mpool.tile(
                        [1, 2, GB, NCH], f32, name="carry", tag="carry"
                    )
                    nc.vector.memset(carry[:, 0, :, 0:1], 0.0)
                    nc.vector.tensor_sub(
                        carry[:, 0, :, 1:NCH], r[:, 0, :, 0:NCH - 1],
                        r[:, 1, :, 0:NCH - 1],
                    )
                    for i in range(GB):
                        nc.vector.tensor_scalar(
                            carry[:, 0, i, :], carry[:, 0, i, :],
                            r[0:1, 0, i, NCH - 1:NCH], None, op0=Alu.subtract,
                        )
                    nc.vector.tensor_scalar(
                        carry[:, 1, :, :], carry[:, 0, :, :], -1.0, None,
                        op0=Alu.mult,
                    )
                    nc.tensor.matmul(
                        midp[:, 32:32 + W2], ones_row[:, :],
                        carry[:, :, :, :].rearrange("p a b c -> p (a b c)"),
                        start=True, stop=True,
                    )
                    # stage cumsums in SBUF (2-PSUM-operand tensor ops illegal)
                    cums = mpool.tile(
                        [P, 2, GB, NCH], f32, name="cums", tag="cums"
                    )
                    nc.scalar.activation(
                        cums[:, :, :, :],
                        midp[:, 0:W2].rearrange("p (a b c) -> p a b c",
                                                a=2, b=GB),
                        Act.Copy,
                    )
                    pbc_v = midp[:, 32:32 + W2].rearrange(
                        "p (a b c) -> p a b c", a=2, b=GB
                    )

                    # A and B into ab (series: 0 = A, 2 = B)
                    abA = ab[:, bs, 0, :]
                    abB = ab[:, bs, 2, :]
                    d1 = mpool.tile([P, GB, NCH], f32, name="d1", tag="d1")
                    nc.vector.tensor_sub(
                        d1[:, :, :], cums[:, 0, :, :], cums[:, 1, :, :]
                    )
                    nc.vector.tensor_add(abA, d1[:, :, :], pbc_v[:, 0, :, :])
                    nc.vector.tensor_sub(abA, abA, cum_rhs[:, 0, :, :])
                    nc.vector.tensor_add(abA, abA, scm[:, bs, :, 1])
                    nc.vector.tensor_add(abA, abA, scm[:, bs, :, 0])
                    nc.vector.tensor_sub(abB, pbc_v[:, 1, :, :], d1[:, :, :])
                    nc.vector.tensor_sub(abB, abB, scm[:, bs, :, 1])
                    nc.vector.tensor_add(abB, abB, scm[:, bs, :, 2])

                    # ---- phase 3 (per batch): B broadcast + fused span ----
                    # A[s] is indexed by the PARTITION of a span tile: used
                    # directly as a per-partition scalar. B[e] is indexed by
                    # the free dim: transpose chunks to a row, broadcast
                    # across partitions with a K=1 ones matmul. Span chunk =
                    # ONE fused op (DVE tensor_scalar add / ACT Identity+bias)
                    for b in range(b0, b0 + GB):
                        brow = rpool.tile([1, BT], f32, name="brow", tag="brow")
                        for half in range(2):
                            ptrB = rps.tile(
                                [1, 512], f32, name="ptrB", tag="ptrB"
                            )
                            for q in range(4):
                                c = half * 4 + q
                                nc.tensor.transpose(
                                    ptrB[0:1, q * P:(q + 1) * P],
                                    ab[:, b, 2:3, c], ident_sb[:, :],
                                )
                            nc.vector.tensor_copy(
                                brow[0:1, half * 512:(half + 1) * 512],
                                ptrB[:, :],
                            )
                        bbc = sps.tile([P, BT], f32)   # B broadcast, 2 banks
                        for half in range(2):
                            nc.tensor.matmul(
                                bbc[:, half * 512:(half + 1) * 512],
                                ones_row[:, :],
                                brow[0:1, half * 512:(half + 1) * 512],
                                start=True, stop=True,
                            )

                        for c in range(NCH):
                            osb = osb_t[b % 2][c]
                            off = c * P
                            a_col = ab[:, b, 0, c:c + 1]   # [P, 1] scalar
                            e0 = c * P
                            n = 0
                            while e0 < BT:
                                w = min(512, BT - e0)
                                o0 = e0 - off
                                if (c + n) % 2:
                                    nc.vector.tensor_scalar(
                                        osb[:, o0:o0 + w], bbc[:, e0:e0 + w],
                                        a_col, None, op0=Alu.add,
                                    )
                                else:
                                    nc.scalar.activation(
                                        osb[:, o0:o0 + w], bbc[:, e0:e0 + w],
                                        Act.Identity, bias=a_col,
                                    )
                                e0 += w
                                n += 1
                            # restore -1000 in the diag block's lower part
                            d0 = c * P - off
                            nc.vector.copy_predicated(
                                osb[:, d0:d0 + P], trili_sb[:, :],
                                neg128[:, :],
                            )
                            # store (full width for c<4, right part for c>=4)
                            eng = out_q[(b * NCH + c) % 8]
                            eng.dma_start(
                                out=out_d[b, c * P:(c + 1) * P, off:BT],
                                in_=osb[:, :],
                            )
    nc.compile()
    return nc


def make_in_maps(x, sen_vecs, sen_mask, Wb, bb, Wi, bi, Wo, bo):
    """Shard full inputs into per-core input maps (data-parallel over batch)."""
    f = np.float32
    x = np.asarray(x, f)
    sen_vecs = np.asarray(sen_vecs, f)
    sen_mask = np.asarray(sen_mask, np.int32)
    Wb, Wi, Wo = (np.asarray(a, f) for a in (Wb, Wi, Wo))
    b3 = np.stack([np.asarray(bb, f), np.asarray(bi, f), np.asarray(bo, f)], axis=1)
    b3 = np.ascontiguousarray(b3)
    triu = np.triu(np.ones((P, P), f))
    trili = np.tril(np.ones((P, P), np.int32), -1)
    ident = np.eye(P, dtype=f)

    in_maps = []
    for i in range(N_CORES):
        b0, b1 = i * B_LOC, (i + 1) * B_LOC
        in_maps.append({
            "svt": np.ascontiguousarray(sen_vecs[b0:b1].transpose(0, 2, 1)),
            "xT": np.ascontiguousarray(x[b0:b1].T),
            "Wb": Wb, "Wi": Wi, "Wo": Wo,
            "b3": b3,
            "mask": np.ascontiguousarray(sen_mask[b0:b1]),
            "triu": triu,
            "trili": trili,
            "ident": ident,
        })
    return in_maps


_CACHE = {}


def kernel(x, sen_vecs, sen_mask, Wb, bb, Wi, bi, Wo, bo):
    _import_concourse()
    from concourse.bass_utils import run_bass_kernel_spmd

    if "nc" not in _CACHE:
        _CACHE["nc"] = build_bass()
    nc = _CACHE["nc"]
    in_maps = make_in_maps(x, sen_vecs, sen_mask, Wb, bb, Wi, bi, Wo, bo)
    res = run_bass_kernel_spmd(nc, in_maps, list(range(N_CORES)))
    return np.concatenate([r["out"] for r in res.results], axis=0)
